# revision 26
# baseline (speedup 1.0000x reference)
"""TRN2 Bass kernel for nn_Attention_87308095193383.

Sharding: 8 cores = (batch b in 0..3) x (query-half h in 0..1).
Each core computes, for its batch:
  - conv1/conv2 + GroupNorm fully (stats need full N),
  - pe_attn^T slice [m=2048, n=1024] for its query half,
  - k,v fully; q for its half,
  - attention + proj for its half -> outT [512, 1024].
Host permutes the N columns per core so "my queries" are always columns
0:1024 of the device program (SPMD: one program, per-core data).

v2 schedule: conv (bf16) runs first; k/v/pe_attn are woven into the
attention window as PE filler work so the DVE mask-multiply stream
(the critical engine) starts ~45us in and never starves.  Attention
operands are bf16; softmax denominator comes from a ones column in v
(M=65 av matmuls), drained per (nq,hp) via ACT copy + DVE recip +
gpsimd broadcast/divide.  proj bias is folded in as a K=1 matmul row.
SBUF is tight: pe_sb aliases kT_sb and cw1/cw2 alias o_sb.
"""
import numpy as np
import ml_dtypes

import concourse.bass as bass
import concourse.mybir as mybir
import concourse.tile as tile
from concourse import bacc
from concourse.bass_utils import run_bass_kernel_spmd

F32R = mybir.dt.float32r
F32 = mybir.dt.float32
BF16 = mybir.dt.bfloat16
AF = mybir.ActivationFunctionType
ALU = mybir.AluOpType

N_CORES = 8
C = 512          # channels
CT = C // 128    # 4 c-tiles
N = 2048         # sequence length
NT = N // 128    # 16 m-tiles
NQ = 1024        # queries per core
H = 8            # heads
D = 64           # head dim
SCALE = D ** -0.5
EPS = 1e-5
GROUPS = 8       # 2 groups per 128-partition tile (64 ch/group)


def build():
    nc = bacc.Bacc("TRN2", target_bir_lowering=False, debug=False,
                   num_devices=N_CORES)

    def din(name, shape, dt=F32R):
        return nc.dram_tensor(name, shape, dt, kind="ExternalInput").ap()

    peT = din("peT", [C, N])
    xT = din("xT", [C, N], BF16)
    cw1 = din("cw1", [C, C])           # conv1_w.T  [c_in, o]
    cw2 = din("cw2", [C, C])
    qw = din("qw", [C, 3 * C], BF16)   # qkv_w.T  [c_in, o]
    pw = din("pw", [C, C])             # proj_w.T
    pbr = din("pbr", [1, C])           # proj bias as a row
    gnp = din("gnp", [6, C], F32)      # cb1,cb2,gn1g,gn1b,gn2g,gn2b packed
    gmask_in = din("gmask", [128, 2], F32)
    gmaskT_in = din("gmaskT", [2, 128], F32)
    vones_in = din("vones", [128, NT * H], BF16)
    outT = nc.dram_tensor("outT", [C, NQ], F32, kind="ExternalOutput").ap()

    with tile.TileContext(nc) as tc:
        _build_body(nc, tc, peT, xT, cw1, cw2, qw, pw, gnp,
                    pbr, gmask_in, gmaskT_in, vones_in, outT)
    nc.compile()
    return nc


def _build_body(nc, tc, peT, xT, cw1, cw2, qw, pw, gnp,
                pbr, gmask_in, gmaskT_in, vones_in, outT):
    from contextlib import ExitStack
    ctx = ExitStack()
    with ctx:
        consts = ctx.enter_context(tc.tile_pool(name="consts", bufs=1))
        work = ctx.enter_context(tc.tile_pool(name="work", bufs=1))

        # ---- constants (tiles now; DMAs issued after the big inputs)
        gmask = consts.tile([128, 2], F32)     # group-membership mask
        gmaskT = consts.tile([2, 128], F32)
        epst = consts.tile([128, 1], F32)
        nc.vector.memset(epst, EPS)
        ones1_f = consts.tile([1, 512], F32)   # moving row for bias matmul
        nc.vector.memset(ones1_f, 1.0)
        ones1 = ones1_f.bitcast(F32R)
        pbrow = consts.tile([1, C], F32R)
        gnpt = consts.tile([128, 6, CT], F32)
        bias1, bias2 = gnpt[:, 0], gnpt[:, 1]
        g1g, g1b = gnpt[:, 2], gnpt[:, 3]
        g2g, g2b = gnpt[:, 4], gnpt[:, 5]

        # ---- persistent activations / operands
        pa_pool = ctx.enter_context(tc.tile_pool(name="pa", bufs=1))
        pa = pa_pool.tile([128, NT, NQ], BF16)     # sigmoid(pe_attn)^T tiles

        kqv_pool = ctx.enter_context(tc.tile_pool(name="kqv", bufs=1))
        kT_sb = kqv_pool.tile([128, CT, N], F32R)  # doubles as pe_sb in front
        pe_sb = kT_sb
        qT_sb = kqv_pool.tile([128, CT, NQ], F32R)
        v_sb = kqv_pool.tile([128, NT, H, D + 1], BF16)

        x_pool = ctx.enter_context(tc.tile_pool(name="x_pool", bufs=1))
        x_sb = x_pool.tile([128, CT, N], BF16)
        x_r = xT.rearrange("(t p) n -> p t n", p=128)
        qw_sb = x_pool.tile([128, CT, 3 * C], BF16)
        qw_r = qw.rearrange("(t p) o -> p t o", p=128)
        pw_sb = x_pool.tile([128, CT, C], F32R)

        p12_pool = ctx.enter_context(tc.tile_pool(name="p12", bufs=1))
        p1_sb = p12_pool.tile([128, CT, NQ], BF16)
        p2_sb = p12_pool.tile([128, CT, N], BF16)

        o_pool = ctx.enter_context(tc.tile_pool(name="outp", bufs=1))
        o_sb = o_pool.tile([128, CT, NQ], F32R)    # conv weights in front
        cw1_sb = o_sb[:, :, 0:C]
        cw2_sb = o_sb[:, :, C:2 * C]
        # conv inputs lead on all four queues; x/qw/pw follow (needed ~27us
        # later, they load during conv).
        pe_r = peT.rearrange("(t p) n -> p t n", p=128)
        cw1_r = cw1.rearrange("(t p) o -> p t o", p=128)
        cw2_r = cw2.rearrange("(t p) o -> p t o", p=128)
        qs = (nc.sync, nc.scalar, nc.gpsimd, nc.sync)
        for ct_i, eng in enumerate(qs):
            eng.dma_start(pe_sb[:, ct_i], pe_r[:, ct_i])
        nc.scalar.dma_start(cw1_sb[:, 0:2], cw1_r[:, 0:2])
        nc.gpsimd.dma_start(cw1_sb[:, 2:4], cw1_r[:, 2:4])
        nc.scalar.dma_start(cw2_sb[:, 0:2], cw2_r[:, 0:2])
        nc.gpsimd.dma_start(cw2_sb[:, 2:4], cw2_r[:, 2:4])
        for ct_i, eng in enumerate(qs):
            eng.dma_start(x_sb[:, ct_i], x_r[:, ct_i])
            eng.dma_start(qw_sb[:, ct_i], qw_r[:, ct_i])
        nc.gpsimd.dma_start(pw_sb, pw.rearrange("(t p) o -> p t o", p=128))
        nc.sync.dma_start(
            v_sb[:, :, :, D:D + 1].rearrange("p t o u -> p (t o u)"),
            vones_in)
        nc.scalar.dma_start(gnpt, gnp.rearrange("a (t p) -> p a t", p=128))
        nc.scalar.dma_start(gmask, gmask_in)
        nc.scalar.dma_start(gmaskT, gmaskT_in)
        nc.scalar.dma_start(pbrow, pbr)

        # ================= stage A/B: conv + groupnorm =================
        ps_front = tc.tile_pool(name="ps_front", bufs=4, space="PSUM")
        ps_mm = ps_front.__enter__()

        convs = [(cw1_sb, bias1, g1g, g1b, p1_sb, NQ),
                 (cw2_sb, bias2, g2g, g2b, p2_sb, N)]
        stack3s = []
        # pass 1: matmuls + stats for both convs (dense PE stream; the GN
        # scalar chains run afterwards so they never stall the PE).
        for conv_i, (cwsb, cbt, gg, gb, dst, keep) in enumerate(convs):
            stats = work.tile([128, CT, N // 512, 6], F32,
                              tag="gnstats", name=f"stats{conv_i}")
            mv2 = work.tile([128, 2, CT], F32, tag=f"gnmv{conv_i}",
                            name=f"mv2_{conv_i}")
            stack3 = work.tile([128, 3, CT], F32, tag=f"gnstack{conv_i}",
                               name=f"stack3_{conv_i}")
            stack3s.append((stack3, mv2, stats))
            for ot in range(CT):
                for nch in range(N // 512):
                    ps = ps_mm.tile([128, 512], F32, tag="mm")
                    for ct_i in range(CT):
                        nc.tensor.matmul(
                            ps, cwsb[:, ct_i, ot * 128:(ot + 1) * 128],
                            pe_sb[:, ct_i, nch * 512:(nch + 1) * 512],
                            start=(ct_i == 0), stop=(ct_i == CT - 1))
                    nc.vector.bn_stats(stats[:, ot, nch], ps)
                    if nch * 512 < keep:
                        nc.scalar.copy(dst[:, ot, nch * 512:(nch + 1) * 512],
                                       ps)
                nc.vector.bn_aggr(mv2[:, :, ot], stats[:, ot])

        # ---- q matmuls (independent of GN; keeps PE busy while the GN
        # scalar chains run on DVE/ACT)
        for ot in range(CT):
            for nch in range(NQ // 512):
                ps = ps_mm.tile([128, 512], F32, tag="mm")
                for ct_i in range(CT):
                    nc.tensor.matmul(
                        ps, qw_sb[:, ct_i, ot * 128:(ot + 1) * 128],
                        x_sb[:, ct_i, nch * 512:(nch + 1) * 512],
                        start=(ct_i == 0), stop=(ct_i == CT - 1))
                nc.scalar.copy(qT_sb[:, ot, nch * 512:(nch + 1) * 512], ps)

        sc_list = []
        for conv_i, (cwsb, cbt, gg, gb, dst, keep) in enumerate(convs):
            stack3, mv2, stats = stack3s[conv_i]
            nc.vector.tensor_add(stack3[:, 0], mv2[:, 0], cbt)
            nc.vector.tensor_copy(stack3[:, 1], mv2[:, 1])
            nc.vector.tensor_mul(stack3[:, 2], stack3[:, 0], stack3[:, 0])
            # group sums over 64-partition halves (all ots at once)
            gs = ps_mm.tile([2, 3, CT], F32, tag="mm")
            nc.tensor.matmul(gs, gmask, stack3.rearrange("p a t -> p (a t)"),
                             start=True, stop=True)
            gss = work.tile([2, 3, CT], F32, tag=f"gss{conv_i}", name=f"gss{conv_i}")
            nc.scalar.copy(gss, gs)
            gstat = work.tile([2, 2, CT], F32, tag=f"gstat{conv_i}", name=f"gstat{conv_i}")  # [mean, rstd]
            nc.vector.tensor_scalar_mul(gstat[:, 0], gss[:, 0], 1.0 / 64.0)
            vt = work.tile([2, 2, CT], F32, tag=f"gvtmp{conv_i}", name=f"vt{conv_i}")
            nc.vector.tensor_add(vt[:, 0], gss[:, 1], gss[:, 2])
            nc.vector.tensor_scalar_mul(vt[:, 0], vt[:, 0], 1.0 / 64.0)
            nc.vector.tensor_mul(vt[:, 1], gstat[:, 0], gstat[:, 0])
            nc.vector.tensor_sub(vt[:, 0], vt[:, 0], vt[:, 1])
            nc.scalar.activation(vt[:, 0], vt[:, 0], AF.Sqrt, bias=epst[0:2])
            nc.vector.reciprocal(gstat[:, 1], vt[:, 0])
            # broadcast group [mean, rstd] to partitions via indicator MM
            bc_ps = ps_mm.tile([128, 2, CT], F32, tag="mm")
            nc.tensor.matmul(bc_ps, gmaskT,
                             gstat.rearrange("p a t -> p (a t)"),
                             start=True, stop=True)
            bcst = work.tile([128, 2, CT], F32, tag=f"gbc{conv_i}", name=f"bcst{conv_i}")
            nc.scalar.copy(bcst, bc_ps)
            # per-channel affine: y = x*sc + sh
            sc = work.tile([128, 2, CT], F32, tag=f"gsc{conv_i}")
            nc.vector.tensor_mul(sc[:, 0], bcst[:, 1], gg)
            nc.vector.tensor_sub(sc[:, 1], cbt, bcst[:, 0])
            nc.vector.tensor_mul(sc[:, 1], sc[:, 1], sc[:, 0])
            nc.vector.tensor_add(sc[:, 1], sc[:, 1], gb)
            sc_list.append(sc)

        # normalize p1 fully (needed by every pe_attn matmul)
        sc1, sc2 = sc_list
        for ot in range(CT):
            nc.vector.tensor_scalar(
                p1_sb[:, ot], p1_sb[:, ot],
                sc1[:, 0, ot:ot + 1], sc1[:, 1, ot:ot + 1],
                op0=ALU.mult, op1=ALU.add)
        # normalize p2 in 512-col chunks, chunk 0 first (ungates pe mt0/1)
        p2norm_done = [False] * (N // 512)

        def p2norm(chunk):
            if p2norm_done[chunk]:
                return
            p2norm_done[chunk] = True
            s = slice(chunk * 512, (chunk + 1) * 512)
            for ot in range(CT):
                nc.vector.tensor_scalar(
                    p2_sb[:, ot, s], p2_sb[:, ot, s],
                    sc2[:, 0, ot:ot + 1], sc2[:, 1, ot:ot + 1],
                    op0=ALU.mult, op1=ALU.add)

        for _c in range(N // 512):
            p2norm(_c)

        ps_front.__exit__(None, None, None)

        # ================= attention-phase pools =================
        ps_s_ctx = tc.tile_pool(name="ps_s", bufs=2, space="PSUM")
        ps_s = ps_s_ctx.__enter__()
        ps_u_ctx = tc.tile_pool(name="ps_u", bufs=2, space="PSUM")
        ps_u = ps_u_ctx.__enter__()

        # ---- filler emitters (PE work woven into the attention stream) ----
        def emit_k_chunk(chunk):
            # kT columns [chunk*512, (chunk+1)*512], all 4 ot tiles
            s = slice(chunk * 512, (chunk + 1) * 512)
            for op in range(CT // 2):
                slot = ps_s.tile([128, 2, 512], F32, tag="s")
                for j in range(2):
                    ot = 2 * op + j
                    for ct_i in range(CT):
                        nc.tensor.matmul(
                            slot[:, j],
                            qw_sb[:, ct_i, C + ot * 128:C + (ot + 1) * 128],
                            x_sb[:, ct_i, s],
                            start=(ct_i == 0), stop=(ct_i == CT - 1))
                nc.scalar.copy(kT_sb[:, 2 * op:2 * op + 2, s], slot)

        def emit_v_pair(p):
            # v rows for m-tiles (2p, 2p+1)
            slot = ps_s.tile([128, 2, 512], F32, tag="s")
            for j in range(2):
                nt = 2 * p + j
                for ct_i in range(CT):
                    nc.tensor.matmul(
                        slot[:, j], x_sb[:, ct_i, nt * 128:(nt + 1) * 128],
                        qw_sb[:, ct_i, 2 * C:3 * C],
                        start=(ct_i == 0), stop=(ct_i == CT - 1))
            nc.scalar.copy(v_sb[:, 2 * p:2 * p + 2, :, 0:D],
                           slot.rearrange("p a (h d) -> p a h d", h=H))

        def emit_pe_attn(mt):
            # pe_attn^T rows for m-tile mt, both nq chunks, + sigmoid
            p2norm(mt // 4)
            slot = ps_s.tile([128, 2, 512], F32, tag="s")
            for nqc in range(2):
                for ct_i in range(CT):
                    nc.tensor.matmul(
                        slot[:, nqc], p2_sb[:, ct_i, mt * 128:(mt + 1) * 128],
                        p1_sb[:, ct_i, nqc * 512:(nqc + 1) * 512],
                        start=(ct_i == 0), stop=(ct_i == CT - 1))
            # sigmoid(r) = (tanh(r/2)+1)/2: tanh shares the exp table set
            # (no ACT table reload); the +1 runs on gpsimd and the /2 is
            # folded into the exp scale.
            pamt = pa[:, mt].rearrange("p (a n) -> p a n", a=2)
            nc.scalar.activation(pamt, slot, AF.Tanh, scale=0.5)
            nc.vector.tensor_scalar_add(pamt, pamt, 1.0)

        outT_r = outT.rearrange("(t p) n -> p t n", p=128)

        def emit_proj_op(nqc, op):
            s = slice(nqc * 512, (nqc + 1) * 512)
            slot = ps_s.tile([128, 2, 512], F32, tag="s", name="pslot")
            for j in range(2):
                ot = 2 * op + j
                for ct_i in range(CT):
                    nc.tensor.matmul(
                        slot[:, j],
                        pw_sb[:, ct_i, ot * 128:(ot + 1) * 128],
                        o_sb[:, ct_i, s],
                        start=(ct_i == 0), stop=False)
                nc.tensor.matmul(
                    slot[:, j], pbrow[0:1, ot * 128:(ot + 1) * 128],
                    ones1, start=False, stop=True)
            fin = work.tile([128, 2, 512], F32, tag="fin", bufs=2)
            nc.scalar.copy(fin, slot)
            eng = nc.sync if op == 0 else nc.gpsimd
            eng.dma_start(outT_r[:, 2 * op:2 * op + 2, s], fin)

        def emit_proj(nqc):
            for op in range(CT // 2):
                emit_proj_op(nqc, op)

        emit_k_chunk(0)
        emit_v_pair(0)

        # front fillers: earliest-deadline work runs before the attention
        # loop (front is PE-serial anyway; DVE idles there regardless).
        emit_pe_attn(0)
        emit_pe_attn(1)
        emit_k_chunk(1)
        emit_pe_attn(2)
        emit_pe_attn(3)

        # remaining filler queue, ordered by first-use deadline.
        fillers = [("v", 1), ("pe", 4), ("pe", 5), ("pe", 6), ("pe", 7),
                   ("v", 2), ("pe", 8), ("pe", 9), ("k", 2),
                   ("v", 3), ("pe", 10), ("pe", 11),
                   ("v", 4), ("pe", 12), ("pe", 13), ("k", 3),
                   ("v", 5), ("pe", 14), ("pe", 15),
                   ("v", 6), ("v", 7)]

        def run_filler(f):
            kind, arg = f
            if kind == "pe":
                emit_pe_attn(arg)
            elif kind == "k":
                emit_k_chunk(arg)
            else:
                emit_v_pair(arg)

        # ================= attention =================
        filler_i = 0
        for nq in range(2):
            nqs = slice(nq * 512, (nq + 1) * 512)
            for hpp in range(2):          # head-pair pairs: (0,1), (2,3)
                u_slots = {}
                for hp in (2 * hpp, 2 * hpp + 1):
                    u_slots[hp] = ps_u.tile([128, 2, 512], F32, tag="u",
                                            name=f"u{hp}")
                for mt2 in range(8):
                    for hp in (2 * hpp, 2 * hpp + 1):
                        kt = hp
                        mts = (2 * mt2, 2 * mt2 + 1)
                        # fillers: 2 per iteration in the first pass
                        if nq == 0 and hpp == 0:
                            for _ in range(2):
                                if filler_i < len(fillers):
                                    run_filler(fillers[filler_i])
                                    filler_i += 1
                        # proj for nq0: per-op pieces woven into nq1
                        if nq == 1 and hpp == 1 and hp == 2 * hpp and \
                                mt2 in (1, 3):
                            emit_proj_op(0, (mt2 - 1) // 2)
                        sl = ps_s.tile([128, 2, 512], F32, tag="s")
                        sl2 = ps_s.tile([128, 2, 512], F32, tag="s")
                        for j, mt in enumerate(mts):
                            nc.tensor.matmul(
                                sl[:, j],
                                kT_sb[0:64, kt, mt * 128:(mt + 1) * 128],
                                qT_sb[0:64, kt, nqs],
                                start=True, stop=True)
                            nc.tensor.matmul(
                                sl2[:, j],
                                kT_sb[64:128, kt, mt * 128:(mt + 1) * 128],
                                qT_sb[64:128, kt, nqs],
                                start=True, stop=True)
                        t2 = work.tile([128, 2, 2, 512], BF16, tag="t2",
                                       bufs=3)
                        nc.vector.tensor_mul(
                            t2[:, 0], sl,
                            pa[:, 2 * mt2:2 * mt2 + 2, nqs])
                        nc.vector.tensor_mul(
                            t2[:, 1], sl2,
                            pa[:, 2 * mt2:2 * mt2 + 2, nqs])
                        nc.scalar.activation(t2, t2, AF.Exp, scale=SCALE / 2)
                        u = u_slots[hp]
                        for j, mt in enumerate(mts):
                            nc.tensor.matmul(u[0:D + 1, 0],
                                             v_sb[:, mt, 2 * hp, :],
                                             t2[:, 0, j],
                                             start=(mt == 0),
                                             stop=(mt == NT - 1))
                            nc.tensor.matmul(u[0:D + 1, 1],
                                             v_sb[:, mt, 2 * hp + 1, :],
                                             t2[:, 1, j],
                                             start=(mt == 0),
                                             stop=(mt == NT - 1))
                # drain U for this head-pair pair; the final pair goes
                # DVE-direct from PSUM (shorter chain on the kernel tail)
                for hp in (2 * hpp, 2 * hpp + 1):
                    kt = hp
                    u = u_slots[hp]
                    rec = work.tile([1, 2, 512], BF16, tag="rec", bufs=2)
                    if nq == 1 and hpp == 1:
                        with nc.allow_low_precision(
                                reason="bf16 softmax denom"):
                            nc.vector.reciprocal(rec, u[D:D + 1])
                        bc = work.tile([64, 2, 512], BF16, tag="bc", bufs=2)
                        nc.gpsimd.partition_broadcast(bc, rec)
                        nc.vector.tensor_mul(o_sb[0:64, kt, nqs],
                                             u[0:D, 0], bc[:, 0])
                        nc.vector.tensor_mul(o_sb[64:128, kt, nqs],
                                             u[0:D, 1], bc[:, 1])
                        continue
                    ucp = work.tile([D + 1, 2, 512], BF16, tag="ucp", bufs=1)
                    nc.scalar.copy(ucp, u[0:D + 1])
                    with nc.allow_low_precision(reason="bf16 softmax denom"):
                        nc.vector.reciprocal(rec, ucp[D:D + 1])
                    bc = work.tile([64, 2, 512], BF16, tag="bc", bufs=2)
                    nc.gpsimd.partition_broadcast(bc, rec)
                    nc.gpsimd.tensor_mul(o_sb[0:64, kt, nqs],
                                         ucp[0:D, 0], bc[:, 0])
                    nc.gpsimd.tensor_mul(o_sb[64:128, kt, nqs],
                                         ucp[0:D, 1], bc[:, 1])
        emit_proj(1)

        ps_u_ctx.__exit__(None, None, None)
        ps_s_ctx.__exit__(None, None, None)


_NC_CACHE = {}


def _get_nc():
    if "nc" not in _NC_CACHE:
        _NC_CACHE["nc"] = build()
    return _NC_CACHE["nc"]


def make_in_maps(x, pe, qkv_w, proj_w, proj_b, conv1_w, conv1_b, gn1_g, gn1_b,
                 conv2_w, conv2_b, gn2_g, gn2_b):
    f = np.float32
    bf = ml_dtypes.bfloat16
    shared = {
        "cw1": np.ascontiguousarray(np.asarray(conv1_w, f).T),
        "cw2": np.ascontiguousarray(np.asarray(conv2_w, f).T),
        "qw": np.ascontiguousarray(np.asarray(qkv_w, f).T).astype(bf),
        "pw": np.ascontiguousarray(np.asarray(proj_w, f).T),
        "gnp": np.stack([np.asarray(a, f) for a in
                         (conv1_b, conv2_b, gn1_g, gn1_b, gn2_g, gn2_b)]),
        "pbr": np.asarray(proj_b, f).reshape(1, C),
        "gmask": np.repeat(np.eye(2, dtype=f), 64, axis=0),
        "gmaskT": np.ascontiguousarray(
            np.repeat(np.eye(2, dtype=f), 64, axis=0).T),
        "vones": np.ones((128, NT * H), np.float32).astype(bf),
    }
    in_maps = []
    for c in range(N_CORES):
        b, h = c // 2, c % 2
        xT = np.asarray(x[b], f).T
        peT = np.asarray(pe[b], f).T
        if h == 1:
            xT = np.concatenate([xT[:, NQ:], xT[:, :NQ]], axis=1)
            peT = np.concatenate([peT[:, NQ:], peT[:, :NQ]], axis=1)
        m = dict(shared)
        m["xT"] = np.ascontiguousarray(xT).astype(bf)
        m["peT"] = np.ascontiguousarray(peT)
        in_maps.append(m)
    return in_maps


def assemble_out(results):
    B = N_CORES // 2
    out = np.empty((B, N, C), np.float32)
    for c in range(N_CORES):
        b, h = c // 2, c % 2
        out[b, h * NQ:(h + 1) * NQ, :] = results[c]["outT"].T
    return out


PROFILE = False
LAST_RESULT = None


def kernel(**inputs):
    global LAST_RESULT
    nc = _get_nc()
    in_maps = make_in_maps(**inputs)
    r = run_bass_kernel_spmd(nc, in_maps, core_ids=list(range(N_CORES)),
                             trace=PROFILE)
    LAST_RESULT = r
    return assemble_out(r.results)


if __name__ == "__main__":
    nc = build()
    print("build+compile OK")
